# revision 2
# baseline (speedup 1.0000x reference)
"""GCN message-passing kernel for Trainium2, 8 NeuronCores (SPMD).

Math (per reference):
    msg[n]  = sum_{e: dst[e]==n} feature[src[e]]
    h[n]    = msg[n] / deg[n]            (0 if deg==0)
    ge      = relu(h @ W_gc + b_gc)      # [N, 3]
    mult[n] = sum_g (ge[n,g] == max_g ge[n,g])
    out     = (h * mult) @ W_lin.T + b_lin

The kernel is DMA-gather bound (~7 ns per 256 B descriptor on the axon
trn2 cores, independent of queue/packet/call-size), so the design
minimizes gather DESCRIPTOR COUNT.  Two host-side permutations (both
free: the output is un-permuted on the host, and the DRAM feature table
layout is ours) turn the static slot schedule from worst-case bucket
capacities into a near-exact packing:

  * src rows are permuted into 4 ranges of 25,000 rows (int16 gather
    indices address <=32,768 rows) with per-range edge loads balanced to
    250k by out-degree-aware snake dealing.
  * dst nodes are assigned freely to (core, stratum, lane): snake-deal
    by in-degree across cores, then per-core greedy vector bin-packing
    of nodes into 98 strata of 128 lanes so that every (stratum, range)
    bucket's edge count fits its fixed column allocation T[s][r] in
    {2,3} columns of 128 slots (sum_s T[s][r] = 252 per range).

This cuts slots/core from 163,072 (baseline worst-case caps, 30% pad)
to 129,024 (3.2% pad).  The schedule T is a fixed constant, so one
compiled program serves all cores (SPMD) and is cached across calls;
packing failure falls back to wider schedules (256/260/264 cols).

Per-core structure (identical on all 8 cores):
  * 7 superblocks x 4 ranges: one dma_gather call per (sb, range)
    (~4.6k idxs, single_packet=False; idxs live wrapped i->[i%16,i//16]
    in partitions 0-15, replicated to 16-31 for the Q7 tx/rx pair).
  * For each 128-slot column: one-hot [128slot, 128node] via
    vector.tensor_scalar(is_equal) against an iota row, with the
    per-slot target lane as the scalar (pad slots carry -1000 so their
    column is zero; pad gather idxs stride a small hot row region to
    ride open DRAM rows).  TensorE accumulates feat.T @ onehot into
    PSUM msgT[64, 256] per group of 2 strata.
  * Batched epilogue per superblock (14 windows of 128 nodes):
    ge = relu((msg@W_gc)*invdeg + b_gc); mult = #argmax ties;
    out = (msg@W_lin.T) * (invdeg*mult) + b_lin.  msgT is the lhsT for
    both epilogue matmuls, avoiding every transpose.
"""

import sys

sys.path.insert(0, "/opt/trn_rl_repo")

import numpy as np

from concourse import bacc, bass, mybir, tile
from concourse import bass_utils

P = 128
D = 64
WN = 128  # one-hot width / stratum size (nodes)

N_NODES = 100000
N_CORES = 8
NODES_PER_CORE = N_NODES // N_CORES  # 12500

GW = 256  # dst nodes per group (one PSUM accumulator)
GROUPS = (NODES_PER_CORE + GW - 1) // GW  # 49
NODES_PAD = GROUPS * GW  # 12544
SBG = 7  # groups per superblock
SB = GROUPS // SBG  # 7
WINDOWS = 2 * GROUPS  # 98 windows (strata) of 128 nodes
WPB = 2 * SBG  # windows per superblock = 14
STRATA = WINDOWS  # alias: stratum == window
SPB = WPB  # strata per superblock

NR = 4
ROWS_PER_RANGE = 25000
COLS_LADDER = (252, 256, 260, 264)  # per-range column budgets to try

F32 = mybir.dt.float32
I16 = mybir.dt.int16


def build_T(cols):
    """Fixed schedule T[98][4]: columns per (stratum, range), in {2,3},
    per-range sums = cols, balanced per superblock and per stratum."""
    T = np.full((STRATA, NR), 2, dtype=np.int64)
    base, extra = divmod(cols, SB)
    for sb in range(SB):
        rows = slice(sb * SPB, sb * SPB + SPB)
        tot = np.zeros(SPB, dtype=np.int64)
        for r in range(NR):
            ncols = base + (1 if ((sb - r * 2) % SB) < extra else 0)
            n3 = ncols - 2 * SPB
            order = np.argsort(tot, kind="stable")
            T[sb * SPB + order[:n3], r] = 3
            tot = T[rows].sum(axis=1)
    assert T.sum(axis=0).tolist() == [cols] * NR
    assert ((T >= 2) & (T <= 3)).all()
    return T


def _layout(T):
    """Static layout derived from T.

    Returns (seg_cols[sb][r], colstart[sb][r], off_in_seg[s][r], maxc[r],
    tot_cols)."""
    seg_cols = np.zeros((SB, NR), dtype=np.int64)
    off_in_seg = np.zeros((STRATA, NR), dtype=np.int64)
    for sb in range(SB):
        for r in range(NR):
            s0 = sb * SPB
            off_in_seg[s0 : s0 + SPB, r] = np.cumsum(T[s0 : s0 + SPB, r]) - T[
                s0 : s0 + SPB, r
            ]
            seg_cols[sb, r] = T[s0 : s0 + SPB, r].sum()
    colstart = np.zeros((SB, NR), dtype=np.int64)
    run = 0
    for sb in range(SB):
        for r in range(NR):
            colstart[sb, r] = run
            run += seg_cols[sb, r]
    maxc = seg_cols.max(axis=0)
    return seg_cols, colstart, off_in_seg, maxc, int(run)


def build_program(sched, n_reps=1, parts=("gather", "onehot", "mm", "ep")):
    """Build the single-core Bass program (identical across cores).

    sched = (cols, T_flat_tuple).  n_reps > 1 repeats the compute body
    (benchmarking aid: per-rep ns = (T(n) - T(1)) / (n - 1))."""
    cols, T_flat = sched
    T = np.array(T_flat, dtype=np.int64).reshape(STRATA, NR)
    seg_cols, colstart, off_in_seg, maxc, tot_cols = _layout(T)
    tot_slots = tot_cols * P

    nc = bacc.Bacc("TRN2", target_bir_lowering=False, debug=False)

    feat = nc.dram_tensor("feat32", [N_NODES, D], F32, kind="ExternalInput")
    gidx = nc.dram_tensor("gidx", [P, tot_slots // 16], I16, kind="ExternalInput")
    sc = nc.dram_tensor("sc", [P, tot_cols], F32, kind="ExternalInput")
    invdeg = nc.dram_tensor("invdeg", [P, WINDOWS], F32, kind="ExternalInput")
    wgc = nc.dram_tensor("wgc", [D, 3], F32, kind="ExternalInput")
    wlint = nc.dram_tensor("wlint", [D, D], F32, kind="ExternalInput")
    bgc_rep = nc.dram_tensor("bgc_rep", [P, 3 * WPB], F32, kind="ExternalInput")
    blin_rep = nc.dram_tensor("blin_rep", [P, D * SBG], F32, kind="ExternalInput")
    iota32 = nc.dram_tensor("iota32", [P, WN], F32, kind="ExternalInput")
    out = nc.dram_tensor("out", [NODES_PAD, D], F32, kind="ExternalOutput")

    # output viewed as [p, window, d] so a 7-window batch is one DMA
    out_v = out.ap().rearrange("(w p) d -> p w d", p=P)

    with tile.TileContext(nc) as tc:
        with (
            tc.tile_pool(name="const", bufs=1) as cpool,
            tc.tile_pool(name="seg0", bufs=2) as seg0p,
            tc.tile_pool(name="seg1", bufs=2) as seg1p,
            tc.tile_pool(name="seg2", bufs=2) as seg2p,
            tc.tile_pool(name="seg3", bufs=2) as seg3p,
            tc.tile_pool(name="oh", bufs=4) as ohp,
            tc.tile_pool(name="msg", bufs=SBG + 2) as msgp,
            tc.tile_pool(name="eps", bufs=2) as epsp,
            tc.tile_pool(name="outs", bufs=3) as outsp,
            tc.tile_pool(name="agg", bufs=2, space="PSUM") as aggp,
            tc.tile_pool(name="gep", bufs=2, space="PSUM") as gepp,
            tc.tile_pool(name="zp", bufs=2, space="PSUM") as zpp,
        ):
            segps = [seg0p, seg1p, seg2p, seg3p]

            # ---- preload constants into SBUF ----
            gidx_s = cpool.tile([P, tot_slots // 16], I16, tag="gidx")
            nc.sync.dma_start(out=gidx_s[:], in_=gidx.ap())
            sc_s = cpool.tile([P, tot_cols], F32, tag="sc")
            nc.sync.dma_start(out=sc_s[:], in_=sc.ap())
            inv_s = cpool.tile([P, WINDOWS], F32, tag="invdeg")
            nc.sync.dma_start(out=inv_s[:], in_=invdeg.ap())
            wgc_s = cpool.tile([D, 3], F32, tag="wgc")
            nc.sync.dma_start(out=wgc_s[:], in_=wgc.ap())
            wlt_s = cpool.tile([D, D], F32, tag="wlint")
            nc.sync.dma_start(out=wlt_s[:], in_=wlint.ap())
            bgc_s = cpool.tile([P, 3 * WPB], F32, tag="bgc")
            nc.sync.dma_start(out=bgc_s[:], in_=bgc_rep.ap())
            blin_s = cpool.tile([P, D * SBG], F32, tag="blin")
            nc.sync.dma_start(out=blin_s[:], in_=blin_rep.ap())
            iota_s = cpool.tile([P, WN], F32, tag="iota")
            nc.sync.dma_start(out=iota_s[:], in_=iota32.ap())

            for _rep in range(n_reps):
                slot_off = 0  # running slot offset into gidx
                for sb in range(SB):
                    # ---- gather the superblock's edge features (1 call/range)
                    segs = []
                    for r in range(NR):
                        ncols = int(seg_cols[sb, r])
                        nslots = ncols * P
                        seg = segps[r].tile([P, int(maxc[r]), D], F32, tag=f"seg{r}")
                        fview = feat.ap()[
                            r * ROWS_PER_RANGE : (r + 1) * ROWS_PER_RANGE, :
                        ]
                        if "gather" not in parts:
                            segs.append(seg)
                            slot_off += nslots
                            continue
                        nc.gpsimd.dma_gather(
                            out_ap=seg[:, :ncols, :],
                            in_ap=fview,
                            idxs_ap=gidx_s[
                                :, slot_off // 16 : (slot_off + nslots) // 16
                            ],
                            num_idxs=nslots,
                            num_idxs_reg=nslots,
                            elem_size=D,
                            # single-packet coalescing faults the DMA engine
                            # above 64 descriptors/engine; big calls need
                            # per-descriptor packets
                            single_packet=False,
                        )
                        segs.append(seg)
                        slot_off += nslots

                    # ---- aggregate each group: msgT[64, 256] = feat.T @ onehot
                    msgs = []
                    for j in range(SBG):
                        psum = None
                        if "mm" in parts:
                            psum = aggp.tile([D, GW], F32, tag="agg")
                        # PSUM start/stop are per half-region (accumulation
                        # groups bind to regions)
                        for h in range(2):
                            s = sb * SPB + j * 2 + h
                            cols_list = []
                            for r in range(NR):
                                for k in range(int(T[s, r])):
                                    segc = int(off_in_seg[s, r]) + k
                                    cols_list.append((r, segc))
                            nmm = len(cols_list)
                            for ci, (r, segc) in enumerate(cols_list):
                                gcol = int(colstart[sb, r]) + segc
                                oh = None
                                if "onehot" in parts:
                                    oh = ohp.tile([P, WN], F32, tag="oh")
                                    nc.vector.tensor_scalar(
                                        out=oh[:],
                                        in0=iota_s[:],
                                        scalar1=sc_s[:, gcol : gcol + 1],
                                        scalar2=None,
                                        op0=mybir.AluOpType.is_equal,
                                    )
                                if "mm" in parts:
                                    nc.tensor.matmul(
                                        out=psum[:, h * WN : (h + 1) * WN],
                                        lhsT=segs[r][:, segc, :],
                                        rhs=(oh[:] if oh is not None else iota_s[:]),
                                        start=(ci == 0),
                                        stop=(ci == nmm - 1),
                                    )
                        if "mm" in parts:
                            msgT = msgp.tile([D, GW], F32, tag="msg")
                            nc.scalar.copy(out=msgT[:], in_=psum[:])
                            msgs.append(msgT)

                    if "ep" not in parts:
                        continue
                    # ---- epilogue over this superblock's 14 windows ----
                    inv_sb = inv_s[:, sb * WPB : (sb + 1) * WPB]  # [128, 14]
                    inv_b = inv_sb.rearrange("p (w o) -> p w o", o=1).to_broadcast(
                        [P, WPB, 3]
                    )
                    gp = gepp.tile([P, 3 * WPB], F32, tag="gep")
                    for w in range(WPB):
                        j, wi = w // 2, w % 2
                        nc.tensor.matmul(
                            out=gp[:, 3 * w : 3 * w + 3],
                            lhsT=msgs[j][:, wi * P : (wi + 1) * P],
                            rhs=wgc_s[:],
                            start=True,
                            stop=True,
                        )
                    ge_s = epsp.tile([P, 3 * WPB], F32, tag="ge")
                    gp3 = gp[:].rearrange("p (w g) -> p w g", g=3)
                    ge3 = ge_s[:].rearrange("p (w g) -> p w g", g=3)
                    nc.vector.tensor_tensor(
                        out=ge3, in0=gp3, in1=inv_b, op=mybir.AluOpType.mult
                    )
                    nc.vector.tensor_tensor(
                        out=ge_s[:], in0=ge_s[:], in1=bgc_s[:], op=mybir.AluOpType.add
                    )
                    nc.vector.tensor_scalar(
                        out=ge_s[:],
                        in0=ge_s[:],
                        scalar1=0.0,
                        scalar2=None,
                        op0=mybir.AluOpType.max,
                    )
                    top = epsp.tile([P, WPB], F32, tag="top")
                    nc.vector.tensor_reduce(
                        out=top[:],
                        in_=ge3,
                        axis=mybir.AxisListType.X,
                        op=mybir.AluOpType.max,
                    )
                    mask = epsp.tile([P, 3 * WPB], F32, tag="mask")
                    top_b = top[:].rearrange("p (w o) -> p w o", o=1).to_broadcast(
                        [P, WPB, 3]
                    )
                    nc.vector.tensor_tensor(
                        out=mask[:].rearrange("p (w g) -> p w g", g=3),
                        in0=ge3,
                        in1=top_b,
                        op=mybir.AluOpType.is_equal,
                    )
                    mult_t = epsp.tile([P, WPB], F32, tag="mult")
                    nc.vector.tensor_reduce(
                        out=mult_t[:],
                        in_=mask[:].rearrange("p (w g) -> p w g", g=3),
                        axis=mybir.AxisListType.X,
                        op=mybir.AluOpType.add,
                    )
                    q = epsp.tile([P, WPB], F32, tag="q")
                    nc.vector.tensor_tensor(
                        out=q[:], in0=mult_t[:], in1=inv_sb, op=mybir.AluOpType.mult
                    )

                    for half in range(2):
                        zp = zpp.tile([P, D * SBG], F32, tag="zp")
                        for k in range(SBG):
                            w = half * SBG + k
                            j, wi = w // 2, w % 2
                            nc.tensor.matmul(
                                out=zp[:, k * D : (k + 1) * D],
                                lhsT=msgs[j][:, wi * P : (wi + 1) * P],
                                rhs=wlt_s[:],
                                start=True,
                                stop=True,
                            )
                        os_ = outsp.tile([P, D * SBG], F32, tag="outs")
                        qh = (
                            q[:, half * SBG : (half + 1) * SBG]
                            .rearrange("p (w o) -> p w o", o=1)
                            .to_broadcast([P, SBG, D])
                        )
                        nc.vector.tensor_tensor(
                            out=os_[:].rearrange("p (w d) -> p w d", d=D),
                            in0=zp[:].rearrange("p (w d) -> p w d", d=D),
                            in1=qh,
                            op=mybir.AluOpType.mult,
                        )
                        nc.vector.tensor_tensor(
                            out=os_[:], in0=os_[:], in1=blin_s[:], op=mybir.AluOpType.add
                        )
                        w0 = sb * WPB + half * SBG
                        nc.sync.dma_start(
                            out=out_v[:, w0 : w0 + SBG, :],
                            in_=os_[:].rearrange("p (w d) -> p w d", d=D),
                        )

    nc.compile()
    return nc


# ---------------------------------------------------------------- host side


def _src_permute(src):
    """Permute src rows into NR ranges of 25k rows, per-range edge loads
    balanced by out-degree snake dealing.  Returns newrow[old_id]."""
    outdeg = np.bincount(src, minlength=N_NODES)
    order = np.argsort(-outdeg, kind="stable")
    idx = np.arange(N_NODES)
    cyc = idx % (2 * NR)
    b = np.where(cyc < NR, cyc, 2 * NR - 1 - cyc)  # 0123 3210 snake
    newrow = np.empty(N_NODES, dtype=np.int64)
    for r in range(NR):
        rows_r = order[b == r]
        assert len(rows_r) == ROWS_PER_RANGE
        newrow[rows_r] = r * ROWS_PER_RANGE + np.arange(ROWS_PER_RANGE)
    return newrow


def _assign_cores(degv):
    """Snake-deal nodes by total in-degree -> per-core per-range balance."""
    tot = degv.sum(axis=1)
    order = np.argsort(-tot, kind="stable")
    idx = np.arange(N_NODES)
    cyc = idx % (2 * N_CORES)
    c = np.where(cyc < N_CORES, cyc, 2 * N_CORES - 1 - cyc)
    core = np.empty(N_NODES, dtype=np.int64)
    core[order] = c
    return core

def _pack_core(degv_c, T):
    """Greedy vector bin-packing: nodes -> strata.  Returns assign [n]
    (stratum per node) or None on failure."""
    n = len(degv_c)
    R = (T * P).astype(np.float64).copy()
    Tf = (T * P).astype(np.float64)
    C = np.full(STRATA, P, dtype=np.int64)
    tot = degv_c.sum(axis=1)
    order = np.argsort(-tot, kind="stable")
    assign = np.full(n, -1, dtype=np.int64)
    for i in order:
        d = degv_c[i].astype(np.float64)
        ok = (C > 0) & (R >= d).all(axis=1)
        if not ok.any():
            return None
        rel = ((R - d) / Tf).min(axis=1) + 0.0007 * C / P
        rel[~ok] = -np.inf
        s = int(np.argmax(rel))
        assign[i] = s
        R[s] -= d
        C[s] -= 1
    return assign


def host_prep(feature, src, dst, W_gc, b_gc, W_lin, b_lin):
    """Permute + shard + lay out per-core inputs.

    Returns (in_maps, sched, orig_of) where orig_of[c][padded_pos] is the
    original node id at that output row (-1 for pad lanes)."""
    src = np.asarray(src).astype(np.int64)
    dst = np.asarray(dst).astype(np.int64)
    feature = np.asarray(feature, dtype=np.float32)

    # --- permutations
    newrow = _src_permute(src)
    feat32 = np.empty_like(feature)
    feat32[newrow] = feature  # device table row newrow[i] = feature[i]
    src_p = newrow[src]
    r_all = src_p // ROWS_PER_RANGE

    degv = np.zeros((N_NODES, NR), dtype=np.int64)
    np.add.at(degv, (dst, r_all), 1)
    core_of_node = _assign_cores(degv)

    deg = degv.sum(axis=1)
    invd = np.where(deg > 0, 1.0 / np.maximum(deg, 1), 0.0).astype(np.float32)

    # --- per-core packing against the fixed schedule ladder
    sched = None
    strat_of_node = np.full(N_NODES, -1, dtype=np.int64)
    for cols in COLS_LADDER:
        T = build_T(cols)
        ok = True
        for c in range(N_CORES):
            nodes_c = np.where(core_of_node == c)[0]
            a = _pack_core(degv[nodes_c], T)
            if a is None:
                ok = False
                break
            strat_of_node[nodes_c] = a
        if ok:
            sched = (cols, tuple(T.flatten().tolist()))
            break
    assert sched is not None, "packing failed at every schedule in the ladder"
    seg_cols, colstart, off_in_seg, maxc, tot_cols = _layout(T)
    tot_slots = tot_cols * P

    # --- lanes within strata + output map
    lane_of_node = np.empty(N_NODES, dtype=np.int64)
    orig_of = np.full((N_CORES, NODES_PAD), -1, dtype=np.int64)
    for c in range(N_CORES):
        nodes_c = np.where(core_of_node == c)[0]
        s = strat_of_node[nodes_c]
        o = np.argsort(s, kind="stable")
        srt = s[o]
        start = np.zeros(STRATA, dtype=np.int64)
        start[1:] = np.cumsum(np.bincount(srt, minlength=STRATA))[:-1]
        lane = np.arange(len(srt)) - start[srt]
        assert (lane < P).all()
        lane_of_node[nodes_c[o]] = lane
        orig_of[c, srt * P + lane] = nodes_c[o]

    # --- constants (shared across cores)
    iota32 = np.broadcast_to(np.arange(WN, dtype=np.float32), (P, WN)).copy()
    wgc = np.ascontiguousarray(np.asarray(W_gc, dtype=np.float32))
    wlint = np.ascontiguousarray(np.asarray(W_lin, dtype=np.float32).T)
    bgc_rep = np.tile(np.asarray(b_gc, dtype=np.float32).reshape(1, 3), (P, WPB))
    blin_rep = np.tile(np.asarray(b_lin, dtype=np.float32).reshape(1, D), (P, SBG))

    # per-bucket slot starts (same for all cores; schedule is fixed)
    sb_of_s = np.arange(STRATA) // SPB
    bucket_col0 = colstart[sb_of_s[:, None], np.arange(NR)[None, :]] + off_in_seg
    bucket_slot0 = bucket_col0 * P  # [98, 4]

    # --- per-core slot layout
    e_core = core_of_node[dst]
    e_s = strat_of_node[dst]
    e_lane = lane_of_node[dst]
    in_maps = []
    for c in range(N_CORES):
        m = e_core == c
        es, er, elane = e_s[m], r_all[m], e_lane[m]
        esrc = src_p[m] - er * ROWS_PER_RANGE
        key = es * NR + er
        order = np.argsort(key, kind="stable")
        k_sorted = key[order]
        start_of = np.zeros(STRATA * NR, dtype=np.int64)
        start_of[1:] = np.cumsum(np.bincount(k_sorted, minlength=STRATA * NR))[:-1]
        rank = np.arange(k_sorted.size) - start_of[k_sorted]
        slot = bucket_slot0[es[order], er[order]] + rank

        # pad slots stride a small hot row region (open DRAM rows beat
        # cold random reads); valid in every range (1696 < 25000)
        gidx_flat = ((np.arange(tot_slots, dtype=np.int64) * 7) % 1696).astype(
            np.int16
        )
        gidx_flat[slot] = esrc[order].astype(np.int16)
        scv = np.full(tot_slots, -1000.0, dtype=np.float32)
        scv[slot] = elane[order].astype(np.float32)

        # gather idx wrapping: idx i -> [i%16, i//16], replicated into
        # partitions 16-31 for the Q7 tx/rx core pair; remaining rows must
        # still hold valid (>= -1, in-range) values
        gidx_w = np.zeros((P, tot_slots // 16), dtype=np.int16)
        wrapped = gidx_flat.reshape(-1, 16).T
        gidx_w[:16] = wrapped
        gidx_w[16:32] = wrapped
        sc_arr = np.ascontiguousarray(scv.reshape(-1, P).T)

        iv = np.zeros(NODES_PAD, dtype=np.float32)
        valid = orig_of[c] >= 0
        iv[valid] = invd[orig_of[c][valid]]
        invdeg_c = np.ascontiguousarray(iv.reshape(WINDOWS, P).T)

        in_maps.append(
            {
                "feat32": feat32,
                "gidx": gidx_w,
                "sc": sc_arr,
                "invdeg": invdeg_c,
                "wgc": wgc,
                "wlint": wlint,
                "bgc_rep": bgc_rep,
                "blin_rep": blin_rep,
                "iota32": iota32,
            }
        )

    return in_maps, sched, orig_of


_PROGRAM_CACHE = {}


def kernel(**inputs):
    in_maps, sched, orig_of = host_prep(
        inputs["feature"],
        inputs["src"],
        inputs["dst"],
        inputs["W_gc"],
        inputs["b_gc"],
        inputs["W_lin"],
        inputs["b_lin"],
    )
    if sched not in _PROGRAM_CACHE:
        _PROGRAM_CACHE[sched] = build_program(sched)
    nc = _PROGRAM_CACHE[sched]
    res = bass_utils.run_bass_kernel_spmd(nc, in_maps, core_ids=list(range(N_CORES)))
    out_full = np.zeros((N_NODES, D), dtype=np.float32)
    for c in range(N_CORES):
        o = np.asarray(res.results[c]["out"])
        valid = orig_of[c] >= 0
        out_full[orig_of[c][valid]] = o[valid]
    return out_full


# revision 12
# speedup vs baseline: 444.3669x; 444.3669x over previous
"""GCN message-passing kernel for Trainium2, 8 NeuronCores (SPMD).

Math (per reference):
    msg[n]  = sum_{e: dst[e]==n} feature[src[e]]
    h[n]    = msg[n] / deg[n]            (0 if deg==0)
    ge      = relu(h @ W_gc + b_gc)      # [N, 3]
    mult[n] = sum_g (ge[n,g] == max_g ge[n,g])
    out     = (h * mult) @ W_lin.T + b_lin

The kernel is DMA-gather bound (~7 ns per 256 B descriptor on the axon
trn2 cores, independent of queue/packet/call-size), so the design
minimizes gather DESCRIPTOR COUNT.  Two host-side permutations (both
free: the output is un-permuted on the host, and the DRAM feature table
layout is ours) turn the static slot schedule from worst-case bucket
capacities into a near-exact packing:

  * src rows are permuted into 4 ranges of 25,000 rows (int16 gather
    indices address <=32,768 rows) with per-range edge loads balanced to
    250k by out-degree-aware snake dealing.
  * dst nodes are assigned freely to (core, stratum, lane): snake-deal
    by in-degree across cores, then per-core greedy vector bin-packing
    of nodes into 98 strata of 128 lanes so that every (stratum, range)
    bucket's edge count fits its fixed column allocation T[s][r] in
    {2,3} columns of 128 slots (sum_s T[s][r] = 252 per range).

This cuts slots/core from 163,072 (baseline worst-case caps, 30% pad)
to 126,976 (1.6% pad).  The schedule T is a fixed constant, so one
compiled program serves all cores (SPMD) and is cached across calls;
packing failure falls back to wider schedules (252...264 cols).

Measured (axon trn2, n_reps differencing; absolute launch floor drifts
across processes, so A/Bs were same-process):
  * baseline 163,072 slots: ~1.54 ms/rep; this design: ~1.12-1.16 ms.
    ~7.9 ns per gather descriptor; the kernel is ~90% gather-bound.
  * within-bucket src sort: -2.7% (open-DRAM-row hits).
  * partition-major output layout: node-major (w p) d output DMAs cost
    ~12.5k extra 256 B descriptors/rep (~100 us).
  * multi-queue SWDGE (num_swdge_queues>1, queue_num=r): CoreSim-correct
    but queues 1-3 return garbage on this runtime even in isolation
    (probe: single gather on queue 1 wrong, queue 0 right).  Dead end
    here; would be the next big lever if the runtime supported it.
  * seg bufs=3 regressed vs bufs=2 (+difference confounded by drift);
    kept 2.

Per-core structure (identical on all 8 cores):
  * 7 superblocks x 4 ranges: one dma_gather call per (sb, range)
    (~4.6k idxs, single_packet=False; idxs live wrapped i->[i%16,i//16]
    in partitions 0-15, replicated to 16-31 for the Q7 tx/rx pair).
  * For each 128-slot column: one-hot [128slot, 128node] via
    vector.tensor_scalar(is_equal) against an iota row, with the
    per-slot target lane as the scalar (pad slots carry -1000 so their
    column is zero; pad gather idxs stride a small hot row region to
    ride open DRAM rows).  TensorE accumulates feat.T @ onehot into
    PSUM msgT[64, 256] per group of 2 strata.
  * Batched epilogue per superblock (14 windows of 128 nodes):
    ge = relu((msg@W_gc)*invdeg + b_gc); mult = #argmax ties;
    out = (msg@W_lin.T) * (invdeg*mult) + b_lin.  msgT is the lhsT for
    both epilogue matmuls, avoiding every transpose.
"""

import sys

sys.path.insert(0, "/opt/trn_rl_repo")

import numpy as np

from concourse import bacc, bass, mybir, tile
from concourse import bass_utils

P = 128
D = 64
WN = 128  # one-hot width / stratum size (nodes)

N_NODES = 100000
N_CORES = 8
NODES_PER_CORE = N_NODES // N_CORES  # 12500

GW = 256  # dst nodes per group (one PSUM accumulator)
GROUPS = (NODES_PER_CORE + GW - 1) // GW  # 49
NODES_PAD = GROUPS * GW  # 12544
SBG = 7  # groups per superblock
SB = GROUPS // SBG  # 7
WINDOWS = 2 * GROUPS  # 98 windows (strata) of 128 nodes
WPB = 2 * SBG  # windows per superblock = 14
STRATA = WINDOWS  # alias: stratum == window
SPB = WPB  # strata per superblock

NR = 4
ROWS_PER_RANGE = 25000
COLS_LADDER = (248, 252, 256, 260, 264)  # per-range column budgets to try

F32 = mybir.dt.float32
I16 = mybir.dt.int16


def build_T(cols):
    """Fixed schedule T[98][4]: columns per (stratum, range), in {2,3},
    per-range sums = cols, balanced per superblock and per stratum."""
    T = np.full((STRATA, NR), 2, dtype=np.int64)
    base, extra = divmod(cols, SB)
    for sb in range(SB):
        rows = slice(sb * SPB, sb * SPB + SPB)
        tot = np.zeros(SPB, dtype=np.int64)
        for r in range(NR):
            ncols = base + (1 if ((sb - r * 2) % SB) < extra else 0)
            n3 = ncols - 2 * SPB
            order = np.argsort(tot, kind="stable")
            T[sb * SPB + order[:n3], r] = 3
            tot = T[rows].sum(axis=1)
    assert T.sum(axis=0).tolist() == [cols] * NR
    assert ((T >= 2) & (T <= 3)).all()
    return T


def _layout(T):
    """Static layout derived from T.

    Returns (seg_cols[sb][r], colstart[sb][r], off_in_seg[s][r], maxc[r],
    tot_cols)."""
    seg_cols = np.zeros((SB, NR), dtype=np.int64)
    off_in_seg = np.zeros((STRATA, NR), dtype=np.int64)
    for sb in range(SB):
        for r in range(NR):
            s0 = sb * SPB
            off_in_seg[s0 : s0 + SPB, r] = np.cumsum(T[s0 : s0 + SPB, r]) - T[
                s0 : s0 + SPB, r
            ]
            seg_cols[sb, r] = T[s0 : s0 + SPB, r].sum()
    colstart = np.zeros((SB, NR), dtype=np.int64)
    run = 0
    for sb in range(SB):
        for r in range(NR):
            colstart[sb, r] = run
            run += seg_cols[sb, r]
    maxc = seg_cols.max(axis=0)
    return seg_cols, colstart, off_in_seg, maxc, int(run)


def build_program(sched, n_reps=1, parts=("gather", "onehot", "mm", "ep"), queues=1):
    """Build the single-core Bass program (identical across cores).

    sched = (cols, T_flat_tuple).  n_reps > 1 repeats the compute body
    (benchmarking aid: per-rep ns = (T(n) - T(1)) / (n - 1)).
    queues > 1 spreads the 4 per-superblock range gathers across that
    many SWDGE queues (separate physical NRT DMA queues)."""
    cols, T_flat = sched
    T = np.array(T_flat, dtype=np.int64).reshape(STRATA, NR)
    seg_cols, colstart, off_in_seg, maxc, tot_cols = _layout(T)
    tot_slots = tot_cols * P

    nc = bacc.Bacc(
        "TRN2", target_bir_lowering=False, debug=False, num_swdge_queues=queues
    )

    feat = nc.dram_tensor("feat32", [N_NODES, D], F32, kind="ExternalInput")
    gidx = nc.dram_tensor("gidx", [P, tot_slots // 16], I16, kind="ExternalInput")
    sc = nc.dram_tensor("sc", [P, tot_cols], F32, kind="ExternalInput")
    invdeg = nc.dram_tensor("invdeg", [P, WINDOWS], F32, kind="ExternalInput")
    wgc = nc.dram_tensor("wgc", [D, 3], F32, kind="ExternalInput")
    wlint = nc.dram_tensor("wlint", [D, D], F32, kind="ExternalInput")
    bgc_rep = nc.dram_tensor("bgc_rep", [P, 3 * WPB], F32, kind="ExternalInput")
    blin_rep = nc.dram_tensor("blin_rep", [P, D * SBG], F32, kind="ExternalInput")
    iota32 = nc.dram_tensor("iota32", [P, WN], F32, kind="ExternalInput")
    # partition-major output: each 7-window batch DMA writes 1792 B
    # contiguous per partition (vs 256 B chunks for node-major layout,
    # which cost ~12.5k extra 256 B DMA descriptors per rep = ~100 us)
    out = nc.dram_tensor("out", [P, WINDOWS * D], F32, kind="ExternalOutput")
    out_v = out.ap().rearrange("p (w d) -> p w d", d=D)

    with tile.TileContext(nc) as tc:
        with (
            tc.tile_pool(name="const", bufs=1) as cpool,
            tc.tile_pool(name="seg0", bufs=2) as seg0p,
            tc.tile_pool(name="seg1", bufs=2) as seg1p,
            tc.tile_pool(name="seg2", bufs=2) as seg2p,
            tc.tile_pool(name="seg3", bufs=2) as seg3p,
            tc.tile_pool(name="oh", bufs=4) as ohp,
            tc.tile_pool(name="msg", bufs=SBG + 2) as msgp,
            tc.tile_pool(name="eps", bufs=2) as epsp,
            tc.tile_pool(name="outs", bufs=3) as outsp,
            tc.tile_pool(name="agg", bufs=2, space="PSUM") as aggp,
            tc.tile_pool(name="gep", bufs=2, space="PSUM") as gepp,
            tc.tile_pool(name="zp", bufs=2, space="PSUM") as zpp,
        ):
            segps = [seg0p, seg1p, seg2p, seg3p]

            # ---- preload constants into SBUF ----
            gidx_s = cpool.tile([P, tot_slots // 16], I16, tag="gidx")
            nc.sync.dma_start(out=gidx_s[:], in_=gidx.ap())
            sc_s = cpool.tile([P, tot_cols], F32, tag="sc")
            nc.sync.dma_start(out=sc_s[:], in_=sc.ap())
            inv_s = cpool.tile([P, WINDOWS], F32, tag="invdeg")
            nc.sync.dma_start(out=inv_s[:], in_=invdeg.ap())
            wgc_s = cpool.tile([D, 3], F32, tag="wgc")
            nc.sync.dma_start(out=wgc_s[:], in_=wgc.ap())
            wlt_s = cpool.tile([D, D], F32, tag="wlint")
            nc.sync.dma_start(out=wlt_s[:], in_=wlint.ap())
            bgc_s = cpool.tile([P, 3 * WPB], F32, tag="bgc")
            nc.sync.dma_start(out=bgc_s[:], in_=bgc_rep.ap())
            blin_s = cpool.tile([P, D * SBG], F32, tag="blin")
            nc.sync.dma_start(out=blin_s[:], in_=blin_rep.ap())
            iota_s = cpool.tile([P, WN], F32, tag="iota")
            nc.sync.dma_start(out=iota_s[:], in_=iota32.ap())

            for _rep in range(n_reps):
                slot_off = 0  # running slot offset into gidx
                for sb in range(SB):
                    # ---- gather the superblock's edge features (1 call/range)
                    segs = []
                    for r in range(NR):
                        ncols = int(seg_cols[sb, r])
                        nslots = ncols * P
                        seg = segps[r].tile([P, int(maxc[r]), D], F32, tag=f"seg{r}")
                        fview = feat.ap()[
                            r * ROWS_PER_RANGE : (r + 1) * ROWS_PER_RANGE, :
                        ]
                        if "gather" not in parts:
                            segs.append(seg)
                            slot_off += nslots
                            continue
                        nc.gpsimd.dma_gather(
                            out_ap=seg[:, :ncols, :],
                            in_ap=fview,
                            idxs_ap=gidx_s[
                                :, slot_off // 16 : (slot_off + nslots) // 16
                            ],
                            num_idxs=nslots,
                            num_idxs_reg=nslots,
                            elem_size=D,
                            # single-packet coalescing faults the DMA engine
                            # above 64 descriptors/engine; big calls need
                            # per-descriptor packets
                            single_packet=False,
                            queue_num=r % queues,
                        )
                        segs.append(seg)
                        slot_off += nslots

                    # ---- aggregate each group: msgT[64, 256] = feat.T @ onehot
                    msgs = []
                    for j in range(SBG):
                        psum = None
                        if "mm" in parts:
                            psum = aggp.tile([D, GW], F32, tag="agg")
                        # PSUM start/stop are per half-region (accumulation
                        # groups bind to regions)
                        for h in range(2):
                            s = sb * SPB + j * 2 + h
                            cols_list = []
                            for r in range(NR):
                                for k in range(int(T[s, r])):
                                    segc = int(off_in_seg[s, r]) + k
                                    cols_list.append((r, segc))
                            nmm = len(cols_list)
                            for ci, (r, segc) in enumerate(cols_list):
                                gcol = int(colstart[sb, r]) + segc
                                oh = None
                                if "onehot" in parts:
                                    oh = ohp.tile([P, WN], F32, tag="oh")
                                    nc.vector.tensor_scalar(
                                        out=oh[:],
                                        in0=iota_s[:],
                                        scalar1=sc_s[:, gcol : gcol + 1],
                                        scalar2=None,
                                        op0=mybir.AluOpType.is_equal,
                                    )
                                if "mm" in parts:
                                    nc.tensor.matmul(
                                        out=psum[:, h * WN : (h + 1) * WN],
                                        lhsT=segs[r][:, segc, :],
                                        rhs=(oh[:] if oh is not None else iota_s[:]),
                                        start=(ci == 0),
                                        stop=(ci == nmm - 1),
                                    )
                        if "mm" in parts:
                            msgT = msgp.tile([D, GW], F32, tag="msg")
                            nc.scalar.copy(out=msgT[:], in_=psum[:])
                            msgs.append(msgT)

                    if "ep" not in parts:
                        continue
                    # ---- epilogue over this superblock's 14 windows ----
                    inv_sb = inv_s[:, sb * WPB : (sb + 1) * WPB]  # [128, 14]
                    inv_b = inv_sb.rearrange("p (w o) -> p w o", o=1).to_broadcast(
                        [P, WPB, 3]
                    )
                    gp = gepp.tile([P, 3 * WPB], F32, tag="gep")
                    for w in range(WPB):
                        j, wi = w // 2, w % 2
                        nc.tensor.matmul(
                            out=gp[:, 3 * w : 3 * w + 3],
                            lhsT=msgs[j][:, wi * P : (wi + 1) * P],
                            rhs=wgc_s[:],
                            start=True,
                            stop=True,
                        )
                    ge_s = epsp.tile([P, 3 * WPB], F32, tag="ge")
                    gp3 = gp[:].rearrange("p (w g) -> p w g", g=3)
                    ge3 = ge_s[:].rearrange("p (w g) -> p w g", g=3)
                    nc.vector.tensor_tensor(
                        out=ge3, in0=gp3, in1=inv_b, op=mybir.AluOpType.mult
                    )
                    nc.vector.tensor_tensor(
                        out=ge_s[:], in0=ge_s[:], in1=bgc_s[:], op=mybir.AluOpType.add
                    )
                    nc.vector.tensor_scalar(
                        out=ge_s[:],
                        in0=ge_s[:],
                        scalar1=0.0,
                        scalar2=None,
                        op0=mybir.AluOpType.max,
                    )
                    top = epsp.tile([P, WPB], F32, tag="top")
                    nc.vector.tensor_reduce(
                        out=top[:],
                        in_=ge3,
                        axis=mybir.AxisListType.X,
                        op=mybir.AluOpType.max,
                    )
                    mask = epsp.tile([P, 3 * WPB], F32, tag="mask")
                    top_b = top[:].rearrange("p (w o) -> p w o", o=1).to_broadcast(
                        [P, WPB, 3]
                    )
                    nc.vector.tensor_tensor(
                        out=mask[:].rearrange("p (w g) -> p w g", g=3),
                        in0=ge3,
                        in1=top_b,
                        op=mybir.AluOpType.is_equal,
                    )
                    mult_t = epsp.tile([P, WPB], F32, tag="mult")
                    nc.vector.tensor_reduce(
                        out=mult_t[:],
                        in_=mask[:].rearrange("p (w g) -> p w g", g=3),
                        axis=mybir.AxisListType.X,
                        op=mybir.AluOpType.add,
                    )
                    q = epsp.tile([P, WPB], F32, tag="q")
                    nc.vector.tensor_tensor(
                        out=q[:], in0=mult_t[:], in1=inv_sb, op=mybir.AluOpType.mult
                    )

                    for half in range(2):
                        zp = zpp.tile([P, D * SBG], F32, tag="zp")
                        for k in range(SBG):
                            w = half * SBG + k
                            j, wi = w // 2, w % 2
                            nc.tensor.matmul(
                                out=zp[:, k * D : (k + 1) * D],
                                lhsT=msgs[j][:, wi * P : (wi + 1) * P],
                                rhs=wlt_s[:],
                                start=True,
                                stop=True,
                            )
                        os_ = outsp.tile([P, D * SBG], F32, tag="outs")
                        qh = (
                            q[:, half * SBG : (half + 1) * SBG]
                            .rearrange("p (w o) -> p w o", o=1)
                            .to_broadcast([P, SBG, D])
                        )
                        nc.vector.tensor_tensor(
                            out=os_[:].rearrange("p (w d) -> p w d", d=D),
                            in0=zp[:].rearrange("p (w d) -> p w d", d=D),
                            in1=qh,
                            op=mybir.AluOpType.mult,
                        )
                        nc.vector.tensor_tensor(
                            out=os_[:], in0=os_[:], in1=blin_s[:], op=mybir.AluOpType.add
                        )
                        w0 = sb * WPB + half * SBG
                        nc.sync.dma_start(
                            out=out_v[:, w0 : w0 + SBG, :],
                            in_=os_[:].rearrange("p (w d) -> p w d", d=D),
                        )

    nc.compile()
    return nc


# ---------------------------------------------------------------- host side


def _src_permute(src):
    """Permute src rows into NR ranges of 25k rows, per-range edge loads
    balanced by out-degree snake dealing.  Returns newrow[old_id]."""
    outdeg = np.bincount(src, minlength=N_NODES)
    order = np.argsort(-outdeg, kind="stable")
    idx = np.arange(N_NODES)
    cyc = idx % (2 * NR)
    b = np.where(cyc < NR, cyc, 2 * NR - 1 - cyc)  # 0123 3210 snake
    newrow = np.empty(N_NODES, dtype=np.int64)
    for r in range(NR):
        rows_r = order[b == r]
        assert len(rows_r) == ROWS_PER_RANGE
        newrow[rows_r] = r * ROWS_PER_RANGE + np.arange(ROWS_PER_RANGE)
    return newrow


def _assign_cores(degv):
    """Greedy LPT: biggest nodes first, each to the core whose max
    per-range load stays smallest -> near-perfect core x range balance
    (spread < 3 edges), which is what lets the tight schedules pack."""
    tot = degv.sum(axis=1)
    order = np.argsort(-tot, kind="stable")
    loads = np.zeros((N_CORES, NR))
    counts = np.zeros(N_CORES, dtype=np.int64)
    core = np.empty(N_NODES, dtype=np.int64)
    for i in order:
        d = degv[i]
        m = (loads + d).max(axis=1) + loads.sum(axis=1) * 1e-9
        m[counts >= NODES_PER_CORE] = np.inf
        c = int(np.argmin(m))
        core[i] = c
        loads[c] += d
        counts[c] += 1
    return core

def _pack_core(degv_c, T):
    """Greedy vector bin-packing: nodes -> strata.  Returns assign [n]
    (stratum per node) or None on failure."""
    n = len(degv_c)
    R = (T * P).astype(np.float64).copy()
    Tf = (T * P).astype(np.float64)
    C = np.full(STRATA, P, dtype=np.int64)
    tot = degv_c.sum(axis=1)
    order = np.argsort(-tot, kind="stable")
    assign = np.full(n, -1, dtype=np.int64)
    for i in order:
        d = degv_c[i].astype(np.float64)
        ok = (C > 0) & (R >= d).all(axis=1)
        if not ok.any():
            return None
        rel = ((R - d) / Tf).min(axis=1) + 0.0007 * C / P
        rel[~ok] = -np.inf
        s = int(np.argmax(rel))
        assign[i] = s
        R[s] -= d
        C[s] -= 1
    return assign


def host_prep(feature, src, dst, W_gc, b_gc, W_lin, b_lin):
    """Permute + shard + lay out per-core inputs.

    Returns (in_maps, sched, orig_of) where orig_of[c][padded_pos] is the
    original node id at that output row (-1 for pad lanes)."""
    src = np.asarray(src).astype(np.int64)
    dst = np.asarray(dst).astype(np.int64)
    feature = np.asarray(feature, dtype=np.float32)

    # --- permutations
    newrow = _src_permute(src)
    feat32 = np.empty_like(feature)
    feat32[newrow] = feature  # device table row newrow[i] = feature[i]
    src_p = newrow[src]
    r_all = src_p // ROWS_PER_RANGE

    degv = np.zeros((N_NODES, NR), dtype=np.int64)
    np.add.at(degv, (dst, r_all), 1)
    core_of_node = _assign_cores(degv)

    deg = degv.sum(axis=1)
    invd = np.where(deg > 0, 1.0 / np.maximum(deg, 1), 0.0).astype(np.float32)

    # --- per-core packing against the fixed schedule ladder
    sched = None
    strat_of_node = np.full(N_NODES, -1, dtype=np.int64)
    for cols in COLS_LADDER:
        T = build_T(cols)
        ok = True
        for c in range(N_CORES):
            nodes_c = np.where(core_of_node == c)[0]
            a = _pack_core(degv[nodes_c], T)
            if a is None:
                ok = False
                break
            strat_of_node[nodes_c] = a
        if ok:
            sched = (cols, tuple(T.flatten().tolist()))
            break
    assert sched is not None, "packing failed at every schedule in the ladder"
    seg_cols, colstart, off_in_seg, maxc, tot_cols = _layout(T)
    tot_slots = tot_cols * P

    # --- lanes within strata + output map
    lane_of_node = np.empty(N_NODES, dtype=np.int64)
    orig_of = np.full((N_CORES, NODES_PAD), -1, dtype=np.int64)
    for c in range(N_CORES):
        nodes_c = np.where(core_of_node == c)[0]
        s = strat_of_node[nodes_c]
        o = np.argsort(s, kind="stable")
        srt = s[o]
        start = np.zeros(STRATA, dtype=np.int64)
        start[1:] = np.cumsum(np.bincount(srt, minlength=STRATA))[:-1]
        lane = np.arange(len(srt)) - start[srt]
        assert (lane < P).all()
        lane_of_node[nodes_c[o]] = lane
        orig_of[c, srt * P + lane] = nodes_c[o]

    # --- constants (shared across cores)
    iota32 = np.broadcast_to(np.arange(WN, dtype=np.float32), (P, WN)).copy()
    wgc = np.ascontiguousarray(np.asarray(W_gc, dtype=np.float32))
    wlint = np.ascontiguousarray(np.asarray(W_lin, dtype=np.float32).T)
    bgc_rep = np.tile(np.asarray(b_gc, dtype=np.float32).reshape(1, 3), (P, WPB))
    blin_rep = np.tile(np.asarray(b_lin, dtype=np.float32).reshape(1, D), (P, SBG))

    # per-bucket slot starts (same for all cores; schedule is fixed)
    sb_of_s = np.arange(STRATA) // SPB
    bucket_col0 = colstart[sb_of_s[:, None], np.arange(NR)[None, :]] + off_in_seg
    bucket_slot0 = bucket_col0 * P  # [98, 4]

    # --- per-core slot layout
    e_core = core_of_node[dst]
    e_s = strat_of_node[dst]
    e_lane = lane_of_node[dst]
    in_maps = []
    for c in range(N_CORES):
        m = e_core == c
        es, er, elane = e_s[m], r_all[m], e_lane[m]
        esrc = src_p[m] - er * ROWS_PER_RANGE
        # sort by (bucket, src row): slot order within a bucket is free
        # (each slot carries its own one-hot scalar), and ascending gather
        # addresses give the DMA engine open-DRAM-row hits
        key = es * NR + er
        order = np.argsort(key * (1 << 15) + esrc, kind="stable")
        k_sorted = key[order]
        start_of = np.zeros(STRATA * NR, dtype=np.int64)
        start_of[1:] = np.cumsum(np.bincount(k_sorted, minlength=STRATA * NR))[:-1]
        rank = np.arange(k_sorted.size) - start_of[k_sorted]
        slot = bucket_slot0[es[order], er[order]] + rank

        # pad slots stride a small hot row region (open DRAM rows beat
        # cold random reads); valid in every range (1696 < 25000)
        gidx_flat = ((np.arange(tot_slots, dtype=np.int64) * 7) % 1696).astype(
            np.int16
        )
        gidx_flat[slot] = esrc[order].astype(np.int16)
        scv = np.full(tot_slots, -1000.0, dtype=np.float32)
        scv[slot] = elane[order].astype(np.float32)

        # gather idx wrapping: idx i -> [i%16, i//16], replicated into
        # partitions 16-31 for the Q7 tx/rx core pair; remaining rows must
        # still hold valid (>= -1, in-range) values
        gidx_w = np.zeros((P, tot_slots // 16), dtype=np.int16)
        wrapped = gidx_flat.reshape(-1, 16).T
        gidx_w[:16] = wrapped
        gidx_w[16:32] = wrapped
        sc_arr = np.ascontiguousarray(scv.reshape(-1, P).T)

        iv = np.zeros(NODES_PAD, dtype=np.float32)
        valid = orig_of[c] >= 0
        iv[valid] = invd[orig_of[c][valid]]
        invdeg_c = np.ascontiguousarray(iv.reshape(WINDOWS, P).T)

        in_maps.append(
            {
                "feat32": feat32,
                "gidx": gidx_w,
                "sc": sc_arr,
                "invdeg": invdeg_c,
                "wgc": wgc,
                "wlint": wlint,
                "bgc_rep": bgc_rep,
                "blin_rep": blin_rep,
                "iota32": iota32,
            }
        )

    return in_maps, sched, orig_of


_PROGRAM_CACHE = {}


def kernel(**inputs):
    in_maps, sched, orig_of = host_prep(
        inputs["feature"],
        inputs["src"],
        inputs["dst"],
        inputs["W_gc"],
        inputs["b_gc"],
        inputs["W_lin"],
        inputs["b_lin"],
    )
    if sched not in _PROGRAM_CACHE:
        _PROGRAM_CACHE[sched] = build_program(sched)
    nc = _PROGRAM_CACHE[sched]
    res = bass_utils.run_bass_kernel_spmd(nc, in_maps, core_ids=list(range(N_CORES)))
    out_full = np.zeros((N_NODES, D), dtype=np.float32)
    for c in range(N_CORES):
        o = np.asarray(res.results[c]["out"])  # [P, WINDOWS*D] partition-major
        o = o.reshape(P, WINDOWS, D).transpose(1, 0, 2).reshape(NODES_PAD, D)
        valid = orig_of[c] >= 0
        out_full[orig_of[c][valid]] = o[valid]
    return out_full
